# revision 1
# baseline (speedup 1.0000x reference)
"""Trainium2 Bass kernel for C3Net/SchNet-style interaction block.

Reference computation (per molecule b, atom n, neighbor slot m):
  Wfil = ssp(f_ij @ W_f1 + b_f1) @ W_f2 + b_f2, masked
  y    = s @ W_in2f
  agg  = sum_m Wfil[b,n,m,:] * y[b, neighbors[b,n,m], :]
  v    = ssp(agg @ W_f2out + b_f2out) @ W_dense + b_dense
(ssp(x) = softplus(x) - log 2)

Strategy: data-parallel over the 32 molecules, 4 per NeuronCore (8 cores).
Host-side (numpy): shard, project s -> y, gather y by neighbor index with the
mask folded in (pure indexing / layout prep), transpose f_ij to contraction-
major layout, pack for 2-way row-tiled matmuls, fold the "- log 2" shifts of
both shifted-softplus activations into the following layer's bias.

Device pipeline per 1536-edge sub-chunk (all heavy math on device):
  mm1 (PE, 2x row-tiled via tile_position, K=51 incl. bias ones-row)
  -> softplus as Exp then Ln(x+1) (ACT LUT passes; this toolchain ships no
     direct softplus table)
  -> mm2 (PE, bf16)
  -> fused PSUM-exit + b2' bias + gathered-neighbor multiply in one DVE
     scalar_tensor_tensor pass
  -> neighbor reduction fused into the f2out matmul: 12 PSUM-accumulated
     N=512 matmuls per 128-atom super-block (4 m-runs per matmul), DVE fold
  -> second ssp + dense layer batched over atom halves.
The emission is software-pipelined (chunk k+1's mm1/softplus chain is
emitted before chunk k's mm2 phase) so the PE FIFO never blocks fresh mm1
work behind ACT-dependent consumers; all activations resolve from the
single natural_log_exp_and_others ACT table set (no mid-kernel reloads).
"""

import math

import numpy as np
import ml_dtypes

B, N, NN, A, S, F = 32, 256, 48, 128, 50, 128
NCORES = 8
MPC = B // NCORES            # molecules per core
ATOMS = MPC * N              # 1024 atoms per core
E = ATOMS * NN               # 49152 edges per core
SUPER = 128                  # atoms per super-block (output tile)
NSB = ATOMS // SUPER         # 8 super-blocks per core
SUB = 1536                   # edges per sub-chunk (3 PSUM banks)
NSUB_PER_SB = (SUPER * NN) // SUB   # 4 sub-chunks per super-block
NBLK = E // SUB              # 32 sub-chunks per core
M_PER_SUB = SUB // SUPER     # 12 neighbor-slots per sub-chunk

LOG2 = float(math.log(2.0))
BF16 = ml_dtypes.bfloat16

_BUILT = None
_ACT_ALIAS_DONE = False
USE_SOFTPLUS_ALIAS = False


def _ensure_act_alias():
    """Expose the hardware softplus spline (shipped as the generic 'act2'
    slot of the 'softplus_and_others' ACT table set) under the name
    'softplus' so InstActivation(Softplus) lowers to one LUT pass instead
    of an Exp+Ln pair. Creates a patched act-table dir, points walrus at
    it (BASS_ACT_ROOT_JSON_PATH), and patches the client-side table map
    used by the ACT-table-load inserter (also restricting it to one set so
    the table never reloads mid-kernel)."""
    global _ACT_ALIAS_DONE
    if _ACT_ALIAS_DONE:
        return
    import json
    import os
    import tempfile

    import concourse.bacc as bacc
    import concourse.mybir as mybir
    from neuronxcc.driver.Job import Job
    from neuronxcc.driver.jobs.support.FindActInfo import findActInfoFile

    keep = "softplus_and_others" if USE_SOFTPLUS_ALIAS else \
        "natural_log_exp_and_others"

    src_info = findActInfoFile(Job.getPackageDir(), "gen3")
    src_dir = os.path.dirname(src_info)
    dst_dir = tempfile.mkdtemp(prefix="ant_act_tables_")
    for fn in os.listdir(src_dir):
        os.symlink(os.path.join(src_dir, fn), os.path.join(dst_dir, fn))

    if USE_SOFTPLUS_ALIAS:
        info = json.load(open(src_info))
        for s in info["act_func_sets"]:
            if s["name"] == "softplus_and_others":
                s["act"]["softplus"] = s["act"]["act2"]
                prof_name = s["profile_json"]
        for p in ("act_info.json", prof_name):
            dst = os.path.join(dst_dir, p)
            if os.path.islink(dst) or os.path.exists(dst):
                os.unlink(dst)
        prof = json.load(open(os.path.join(src_dir, prof_name)))
        act2_ent = [e for e in prof["profile_meta_data"]
                    if e["func_name"] == "act2_1p"]
        assert act2_ent, prof["profile_meta_data"][:2]
        sp_ent = dict(act2_ent[0])
        sp_ent["func_name"] = "softplus_1p"
        prof["profile_meta_data"].append(sp_ent)
        json.dump(prof, open(os.path.join(dst_dir, prof_name), "w"))
        json.dump(info, open(os.path.join(dst_dir, "act_info.json"), "w"))

    os.environ["BASS_ACT_ROOT_JSON_PATH"] = os.path.join(
        dst_dir, "act_info.json")

    if not getattr(bacc, "_ant_act_tables_patched", False):
        def _patched_tables(arch):
            info = json.load(
                open(os.path.join(dst_dir, "act_info.json")))
            out = {}
            for ent in info["act_func_sets"]:
                if ent["name"] == keep:
                    out[ent["name"]] = {
                        mybir.ActivationFunctionType.from_pwp(v)
                        for v in ent["act"].keys()
                    }
                else:
                    out[ent["name"]] = set()
            return out

        bacc.get_activation_tables = _patched_tables
        bacc._ant_act_tables_patched = True
    _ACT_ALIAS_DONE = True


def _build_program():
    """Build the Bass/Tile program (one SPMD program, same for all 8 cores)."""
    import concourse.bacc as bacc
    import concourse.mybir as mybir
    from concourse import tile

    dt = mybir.dt
    AF = mybir.ActivationFunctionType
    ALU = mybir.AluOpType

    _ensure_act_alias()

    nc = bacc.Bacc("TRN2", target_bir_lowering=False, debug=False)

    f_pack = nc.dram_tensor("f_pack", [NBLK, 115, SUB // 2], dt.bfloat16,
                            kind="ExternalInput")
    y_pack = nc.dram_tensor("y_pack", [128, E], dt.bfloat16,
                            kind="ExternalInput")
    w1pack = nc.dram_tensor("w1pack", [115, F], dt.bfloat16,
                            kind="ExternalInput")
    w2 = nc.dram_tensor("w2", [F, F], dt.bfloat16, kind="ExternalInput")
    wf2o = nc.dram_tensor("wf2o", [F, A], dt.bfloat16, kind="ExternalInput")
    wd = nc.dram_tensor("wd", [A, A], dt.bfloat16, kind="ExternalInput")
    b2p = nc.dram_tensor("b2p", [F, 1], dt.float32, kind="ExternalInput")
    bf2o = nc.dram_tensor("bf2o", [A, 1], dt.float32, kind="ExternalInput")
    bdp = nc.dram_tensor("bdp", [A, 1], dt.float32, kind="ExternalInput")
    vout = nc.dram_tensor("v_out", [A, ATOMS], dt.float32,
                          kind="ExternalOutput")

    with tile.TileContext(nc) as tc:
        with (
            tc.tile_pool(name="wpool", bufs=1) as wp,
            tc.tile_pool(name="fpool", bufs=3) as fpl,
            tc.tile_pool(name="ypool", bufs=3) as ypl,
            tc.tile_pool(name="upool", bufs=3) as upl,
            tc.tile_pool(name="sppool", bufs=3) as spl,
            tc.tile_pool(name="zpool", bufs=3) as zpl,
            tc.tile_pool(name="opool", bufs=2) as opl,
            tc.tile_pool(name="psumh", bufs=1, space="PSUM") as ph1,
            tc.tile_pool(name="psumw", bufs=1, space="PSUM") as pwf,
            tc.tile_pool(name="psumv", bufs=2, space="PSUM") as pv,
        ):
            w1t = wp.tile([115, F], dt.bfloat16)
            nc.sync.dma_start(w1t[:], w1pack[:])
            w2t = wp.tile([F, F], dt.bfloat16)
            nc.gpsimd.dma_start(w2t[:], w2[:])
            b2pt = wp.tile([F, 1], dt.float32)
            nc.gpsimd.dma_start(b2pt[:], b2p[:])
            wf2ot = wp.tile([F, A], dt.bfloat16)
            nc.gpsimd.dma_start(wf2ot[:], wf2o[:])
            wdt = wp.tile([A, A], dt.bfloat16)
            nc.gpsimd.dma_start(wdt[:], wd[:])
            bf2ot = wp.tile([A, 1], dt.float32)
            nc.gpsimd.dma_start(bf2ot[:], bf2o[:])
            bdpt = wp.tile([A, 1], dt.float32)
            nc.gpsimd.dma_start(bdpt[:], bdp[:])
            v1all = wp.tile([A, ATOMS], dt.float32)

            def emit_load_phase(blk):
                """mm1 + softplus chain for one 1536-edge sub-chunk."""
                ft = fpl.tile([115, SUB // 2], dt.bfloat16, tag="f",
                              name=f"ft{blk}")
                nc.sync.dma_start(ft[:], f_pack[blk])
                yt = ypl.tile([128, SUB], dt.bfloat16, tag="y",
                              name=f"yt{blk}")
                nc.gpsimd.dma_start(
                    yt[:], y_pack[:, blk * SUB:(blk + 1) * SUB])
                h1 = ph1.tile([128, SUB], dt.float32, tag="h1",
                              name=f"h1_{blk}")
                # 2-way row-tiled mm1: rows 0:51 / 64:115 hold independent
                # (weights, edge-half) pairs incl. the bias ones-row.
                nc.tensor.matmul(h1[:, 0:512], w1t[0:51, :],
                                 ft[0:51, 0:512],
                                 start=True, stop=True, tile_position=(0, 0))
                nc.tensor.matmul(h1[:, 512:768], w1t[0:51, :],
                                 ft[0:51, 512:768],
                                 start=True, stop=True, tile_position=(0, 0))
                nc.tensor.matmul(h1[:, 768:1024], w1t[64:115, :],
                                 ft[64:115, 0:256],
                                 start=True, stop=True, tile_position=(64, 0))
                nc.tensor.matmul(h1[:, 1024:1536], w1t[64:115, :],
                                 ft[64:115, 256:768],
                                 start=True, stop=True, tile_position=(64, 0))
                # softplus = ln(exp(x) + 1) in two ACT LUT passes.
                u = upl.tile([128, SUB], dt.float16, tag="u",
                             name=f"u{blk}")
                nc.scalar.activation(u[:], h1[:], AF.Exp)
                sp = spl.tile([128, SUB], dt.bfloat16, tag="sp",
                              name=f"sp{blk}")
                nc.scalar.activation(sp[:], u[:], AF.Ln, bias=1.0)
                return yt, (sp, 0)

            def emit_mm2_phase(state):
                """mm2 + fused exit/bias/y-mul + f2out partial matmuls."""
                blk, yt, (sp, off), v1w = state
                sub = blk % NSUB_PER_SB
                wf = pwf.tile([128, SUB], dt.float32, tag="wf",
                              name=f"wf{blk}")
                for j in range(SUB // 512):
                    nc.tensor.matmul(wf[:, j * 512:(j + 1) * 512], w2t[:],
                                     sp[:, off + j * 512:off + (j + 1) * 512],
                                     start=True, stop=True)
                # z = (wfil + b2') * y_nbh — PSUM exit fused on DVE.
                z = zpl.tile([128, SUB], dt.bfloat16, tag="z",
                             name=f"z{blk}")
                nc.vector.scalar_tensor_tensor(
                    z[:], wf[:], b2pt[:], yt[:], op0=ALU.add, op1=ALU.mult)
                # Neighbor reduction fused into f2out: v1w[:, 512] +=
                # W_f2out.T @ z, 4 m-runs per matmul, folded later.
                for j in range(3):
                    gm = sub * 3 + j
                    nc.tensor.matmul(v1w[:], wf2ot[:],
                                     z[:, j * 512:(j + 1) * 512],
                                     start=(gm == 0), stop=(gm == 11))

            def emit_tail(sb, v1w):
                """Fold the 4 m-run partials into the v1 accumulator.
                (PSUM has one DVE read port, so chain with at most one
                PSUM operand per op.)"""
                s1 = opl.tile([A, SUPER], dt.float32, tag="s1",
                              name=f"s1_{sb}")
                nc.vector.tensor_copy(s1[:], v1w[:, 0:128])
                s2 = opl.tile([A, SUPER], dt.float32, tag="s2",
                              name=f"s2_{sb}")
                nc.vector.tensor_add(s2[:], v1w[:, 128:256], s1[:])
                s3 = opl.tile([A, SUPER], dt.float32, tag="s3",
                              name=f"s3_{sb}")
                nc.vector.tensor_add(s3[:], v1w[:, 256:384], s2[:])
                nc.vector.tensor_add(
                    v1all[:, sb * SUPER:(sb + 1) * SUPER],
                    v1w[:, 384:512], s3[:])

            def emit_final(lo, width):
                """Second ssp + dense layer for `width` atom columns."""
                half = lo // 256
                u2 = opl.tile([A, width], dt.float32, tag="u2",
                              name=f"u2h{half}")
                nc.scalar.activation(u2[:], v1all[:, lo:lo + width], AF.Exp,
                                     bias=bf2ot[:])
                v1sp = opl.tile([A, width], dt.bfloat16, tag="v1sp",
                                name=f"v1sph{half}")
                nc.scalar.activation(v1sp[:], u2[:], AF.Ln, bias=1.0)
                vps = pwf.tile([A, width], dt.float32, tag="wf",
                                name=f"vpsh{half}")
                nc.tensor.matmul(vps[:], wdt[:], v1sp[:],
                                 start=True, stop=True)
                ot = opl.tile([A, width], dt.float32, tag="o",
                              name=f"oth{half}")
                nc.vector.tensor_scalar_add(ot[:], vps[:], bdpt[:])
                nc.sync.dma_start(vout[:, lo:lo + width], ot[:])

            # Software pipeline: the mm1/softplus chain of chunk k+1 is
            # emitted before the mm2-phase of chunk k, so the PE FIFO never
            # blocks new mm1 work behind ACT-dependent mm2 consumers.
            pending = None
            pending_tail = None
            v1w = None
            pair = {}
            for blk in range(NBLK):
                sub = blk % NSUB_PER_SB
                sb = blk // NSUB_PER_SB
                if sub == 0:
                    v1w = pv.tile([A, 512], dt.float32, tag="v1",
                                  name=f"v1w{sb}")
                yt, sp = emit_load_phase(blk)
                if pending is not None:
                    emit_mm2_phase(pending)
                    if pending[0] % NSUB_PER_SB == NSUB_PER_SB - 1:
                        pending_tail = (pending[0] // NSUB_PER_SB,
                                        pending[3])
                if pending_tail is not None:
                    emit_tail(*pending_tail)
                    if pending_tail[0] == NSB // 2:
                        emit_final(0, 512)    # sbs 0-3, hides mid-stream
                    elif pending_tail[0] == NSB - 2:
                        emit_final(512, 256)  # sbs 4-5
                    pending_tail = None
                pending = (blk, yt, sp, v1w)
            emit_mm2_phase(pending)
            emit_tail(NSB - 1, v1w)
            emit_final(768, 256)              # sbs 6-7

    nc.finalize()
    return nc


def _get_program():
    global _BUILT
    if _BUILT is None:
        _BUILT = _build_program()
    return _BUILT


def kernel(s, neighbor_mask, neighbors, f_ij,
           W_f1, b_f1, W_f2, b_f2, W_in2f, W_f2out, b_f2out, W_dense,
           b_dense):
    s = np.asarray(s, np.float32)
    neighbor_mask = np.asarray(neighbor_mask, np.float32)
    neighbors = np.asarray(neighbors)
    f_ij = np.asarray(f_ij, np.float32)
    W_f1 = np.asarray(W_f1, np.float32)
    b_f1 = np.asarray(b_f1, np.float32)
    W_f2 = np.asarray(W_f2, np.float32)
    b_f2 = np.asarray(b_f2, np.float32)
    W_in2f = np.asarray(W_in2f, np.float32)
    W_f2out = np.asarray(W_f2out, np.float32)
    b_f2out = np.asarray(b_f2out, np.float32)
    W_dense = np.asarray(W_dense, np.float32)
    b_dense = np.asarray(b_dense, np.float32)

    # Host prep: in2f projection + neighbor gather (indexing) + layout,
    # vectorized across all 8 per-core shards at once.
    y_all = s @ W_in2f                                     # [B, N, F]
    y_nbh = y_all[np.arange(B)[:, None, None], neighbors]  # [B, N, NN, F]
    y_nbh *= neighbor_mask[..., None]

    w1aug = np.concatenate([W_f1, b_f1[None, :]], axis=0)  # [51, F]
    w1pack = np.zeros((115, F), np.float32)
    w1pack[0:S + 1] = w1aug
    w1pack[64:64 + S + 1] = w1aug
    w1pack = w1pack.astype(BF16)
    w2_b = W_f2.astype(BF16)
    wf2o_b = W_f2out.astype(BF16)
    wd_b = W_dense.astype(BF16)
    b2p = (b_f2 - LOG2 * W_f2.sum(axis=0)).astype(np.float32).reshape(F, 1)
    bf2o = b_f2out.astype(np.float32).reshape(A, 1)
    bdp = (b_dense - LOG2 * W_dense.sum(axis=0)).astype(
        np.float32).reshape(A, 1)

    # Edge order per core: (super-block, m, atom-in-super) — see the
    # device program's m-run layout.
    f8 = (f_ij.reshape(NCORES, NSB, SUPER, NN, S)
          .transpose(0, 1, 3, 2, 4).reshape(NCORES, E, S))
    ft8 = np.ascontiguousarray(f8.transpose(0, 2, 1))      # [8, S, E]
    fta8 = np.concatenate(
        [ft8, np.ones((NCORES, 1, E), np.float32)], axis=1)  # [8, 51, E]
    blocks8 = fta8.reshape(NCORES, S + 1, NBLK, SUB)
    f_pack8 = np.zeros((NCORES, NBLK, 115, SUB // 2), BF16)
    f_pack8[:, :, 0:S + 1, :] = blocks8[:, :, :, 0:SUB // 2].transpose(
        0, 2, 1, 3)
    f_pack8[:, :, 64:64 + S + 1, :] = blocks8[:, :, :, SUB // 2:].transpose(
        0, 2, 1, 3)

    y8 = (y_nbh.reshape(NCORES, NSB, SUPER, NN, F)
          .transpose(0, 1, 3, 2, 4).reshape(NCORES, E, F).astype(BF16))
    y_pack8 = np.ascontiguousarray(y8.transpose(0, 2, 1))  # [8, 128, E]

    in_maps = []
    for c in range(NCORES):
        in_maps.append({
            "f_pack": f_pack8[c],
            "y_pack": y_pack8[c],
            "w1pack": w1pack,
            "w2": w2_b,
            "wf2o": wf2o_b,
            "wd": wd_b,
            "b2p": b2p,
            "bf2o": bf2o,
            "bdp": bdp,
        })

    from concourse.bass_utils import run_bass_kernel_spmd

    nc = _get_program()
    res = run_bass_kernel_spmd(nc, in_maps, list(range(NCORES)))

    out = np.empty((B, N, A), np.float32)
    for c in range(NCORES):
        v_c = res.results[c]["v_out"]                    # [A, ATOMS]
        out[c * MPC:(c + 1) * MPC] = np.ascontiguousarray(
            v_c.T).reshape(MPC, N, A)
    return out

